# revision 26
# baseline (speedup 1.0000x reference)
"""Chamfer loss (nn_ChamferLoss) Bass kernel for Trainium2.

Data-parallel over the batch dim: 8 batches, one NeuronCore each. Per core
(one batch, clouds A = X[b].T and B = Y[b].T, each 4096 x 3 fp32):

  The full 4096x4096 squared-distance matrix t[n,m] is produced tile-by-tile
  directly in PSUM by a single matmul per tile whose contraction rows encode
  the whole formula:

      t[n,m] = sum_c (-2 X_c[n]) * Y_c[m]   (cross terms)
             + sum_c (X_c[n]^2) * 1         (||a||^2)
             + sum_c 1 * (Y_c[m]^2)         (||b||^2)

  Matmul dtype ("bf16x", K=30): every row is decomposed into bf16 hi/lo
  pieces (cross terms: all four hi/lo products; squared norms: three bf16
  terms), accumulated in fp32 PSUM, which reproduces fp32 numerics to
  ~1e-5 at full bf16 matmul speed. (Plain fp32 matmuls run at 1/4 rate;
  float32r is 10x less accurate: loss rel err ~7e-4.)

  d2[n] = min_m t[n,m] is a free-axis DVE min-reduce of each [128, 2048]
  PSUM tile (two tiles = 8 banks, double-buffered against the PE).
  d1[m] = min_n t[n,m] comes from a second, symmetric pass with X and Y
  swapped (rebuilding t transposed is cheaper than transposing it).
  The DVE 1x-mode reduce stream is the bottleneck (~266 us/core); the fused
  TENSOR_TENSOR_REDUCE min path that would halve it faults on TRN2 hardware
  (min-reduce ucode; add-reduce works), so direct reduce is the floor.

  Per-pass partial sums (sum over this core's n of d2[n], and of d1[m]) are
  reduced on-device to a [128, 2] tensor; the final scalar mean over the 8
  cores is assembled on the host in float64 and cast to float32.

Measured (8 cores, axon TRN2): loss rel err ~9.6e-6 vs the jax reference.
In-NEFF repeat-loop timing is machine-state dependent (~±15%): the fp16
max-tree scan (scan="tree16e", tree_bufs=4) measures 219-254 us against
250-277 us for the direct DVE min-reduce scan in the same window (~1.1-1.26x).
"""

import numpy as np

B, C, N = 8, 3, 4096
P = 128      # partition width / rows per block
NTILE = 4    # PSUM tiles per row-block (each W = n/NTILE wide)

_cache = {}


def _build(n=N, mm_dtype="float32r", scan="ttr", evac_bufs=4, reps=1, pe_rot=False, ntile=NTILE, tree_bufs=2):
    import concourse.bacc as bacc
    import concourse.mybir as mybir
    from concourse import tile

    f32 = mybir.dt.float32
    f16 = mybir.dt.float16
    AL = mybir.AluOpType
    AX = mybir.AxisListType
    opdt = mybir.dt.float32r if mm_dtype == "float32r" else f32

    bf16 = mybir.dt.bfloat16
    bf16x = mm_dtype == "bf16x"
    K = 30 if bf16x else 9   # contraction rows
    nblk = n // P            # row blocks per pass
    W = n // ntile           # columns per PSUM tile
    PF = (C * n) // P        # flat layout partition count (96 for n=4096)
    nacc = 2 if scan == "ttr" else (
        1 if (scan.startswith("tree16") or scan == "evac16") else ntile
    )
    BIG = 1.0e30

    nc = bacc.Bacc("TRN2", target_bir_lowering=False, debug=False)
    X_d = nc.dram_tensor("X", [C, n], f32, kind="ExternalInput")
    Y_d = nc.dram_tensor("Y", [C, n], f32, kind="ExternalInput")
    out_d = nc.dram_tensor("out", [P, 2], f32, kind="ExternalOutput")

    with tile.TileContext(nc) as tc:
        with (
            tc.tile_pool(name="big", bufs=1) as big,
            tc.tile_pool(name="small", bufs=1) as small,
            tc.tile_pool(name="evac", bufs=evac_bufs) as evac,
            tc.tile_pool(name="tree", bufs=tree_bufs) as tree,
            tc.tile_pool(name="psum", bufs=ntile, space="PSUM") as psum,
        ):
            kdt = bf16 if bf16x else opdt
            lhsT1 = big.tile([K, n], kdt, tag="lhsT1")
            rhs1 = big.tile([K, n], kdt, tag="rhs1")
            lhsT2 = big.tile([K, n], kdt, tag="lhsT2")
            rhs2 = big.tile([K, n], kdt, tag="rhs2")

            flatX = small.tile([PF, P], f32, tag="flatX")
            flatY = small.tile([PF, P], f32, tag="flatY")

            mins1 = small.tile([P, nacc * nblk], f32, tag="mins1")
            mins2 = small.tile([P, nacc * nblk], f32, tag="mins2")
            minb1 = small.tile([P, nblk], f32, tag="minb1")
            minb2 = small.tile([P, nblk], f32, tag="minb2")
            outt = small.tile([P, 2], f32, tag="outt")

            # ---- setup ----
            # flat [3n/128, 128] layout for fast elementwise prep; every
            # operand row group is produced by DVE ops and DMA-reshaped into
            # [3, n] row layout (same linear element order on both sides).
            xf_src = X_d[:].rearrange("c n -> (c n)").rearrange("(p f) -> p f", f=P)
            yf_src = Y_d[:].rearrange("c n -> (c n)").rearrange("(p f) -> p f", f=P)
            nc.sync.dma_start(out=flatX[:], in_=xf_src)
            nc.sync.dma_start(out=flatY[:], in_=yf_src)

            def ft(name, dtype):
                return small.tile([PF, P], dtype, tag=name, name=name)

            def rows(dst, g, src):
                """DMA flat src into row group g (3 rows) of dst."""
                nc.sync.dma_start(out=dst[3 * g : 3 * g + 3, :], in_=src[:])

            if not bf16x:
                # K=9: lhsT = [-2X; X^2; 1], rhs = [Y; 1; Y^2] (and swapped)
                sq, scl, pln = {}, {}, {}
                for nm, flat in (("x", flatX), ("y", flatY)):
                    sq[nm] = ft(f"sq_{nm}", opdt)
                    scl[nm] = ft(f"scl_{nm}", opdt)
                    pln[nm] = ft(f"pln_{nm}", opdt)
                    nc.vector.tensor_tensor(
                        out=sq[nm][:], in0=flat[:], in1=flat[:], op=AL.mult
                    )
                    nc.vector.tensor_scalar_mul(
                        out=scl[nm][:], in0=flat[:], scalar1=-2.0
                    )
                    nc.vector.tensor_scalar_mul(
                        out=pln[nm][:], in0=flat[:], scalar1=1.0
                    )
                onesf = ft("onesf", opdt)
                nc.vector.tensor_scalar(
                    out=onesf[:], in0=flatX[:], scalar1=0.0, scalar2=1.0,
                    op0=AL.mult, op1=AL.add,
                )
                for dst, srcs in (
                    (lhsT1, (scl["x"], sq["x"], onesf)),
                    (rhs1, (pln["y"], onesf, sq["y"])),
                    (lhsT2, (scl["y"], sq["y"], onesf)),
                    (rhs2, (pln["x"], onesf, sq["x"])),
                ):
                    for g, src in enumerate(srcs):
                        rows(dst, g, src)
            else:
                # K=30 bf16 hi/lo decomposition (fp32-accurate):
                #   cross: (-2x)(y) = (mh+ml)(yh+yl), all 4 products
                #   norms: x^2 and y^2 each as 3 bf16 terms
                def split2(flat, scale1, nm):
                    """-> (hi_bf16, lo_bf16) with hi+lo ~== scale1*flat."""
                    base = ft(f"s2b_{nm}", f32)
                    nc.vector.tensor_scalar_mul(
                        out=base[:], in0=flat[:], scalar1=scale1
                    )
                    h = ft(f"s2h_{nm}", bf16)
                    h32 = ft(f"s2h32_{nm}", f32)
                    l = ft(f"s2l_{nm}", bf16)
                    nc.vector.tensor_scalar_mul(out=h[:], in0=base[:], scalar1=1.0)
                    nc.vector.tensor_scalar_mul(out=h32[:], in0=h[:], scalar1=1.0)
                    nc.vector.tensor_tensor(
                        out=l[:], in0=base[:], in1=h32[:], op=AL.subtract
                    )
                    return h, l

                def split3sq(flat, nm):
                    """-> (h, m, l) bf16 with h+m+l ~== flat*flat."""
                    s = ft(f"sq_{nm}", f32)
                    nc.vector.tensor_tensor(out=s[:], in0=flat[:], in1=flat[:], op=AL.mult)
                    h = ft(f"s3h_{nm}", bf16)
                    h32 = ft(f"s3h32_{nm}", f32)
                    d1 = ft(f"s3d1_{nm}", f32)
                    m = ft(f"s3m_{nm}", bf16)
                    m32 = ft(f"s3m32_{nm}", f32)
                    l = ft(f"s3l_{nm}", bf16)
                    nc.vector.tensor_scalar_mul(out=h[:], in0=s[:], scalar1=1.0)
                    nc.vector.tensor_scalar_mul(out=h32[:], in0=h[:], scalar1=1.0)
                    nc.vector.tensor_tensor(out=d1[:], in0=s[:], in1=h32[:], op=AL.subtract)
                    nc.vector.tensor_scalar_mul(out=m[:], in0=d1[:], scalar1=1.0)
                    nc.vector.tensor_scalar_mul(out=m32[:], in0=m[:], scalar1=1.0)
                    nc.vector.tensor_tensor(out=l[:], in0=d1[:], in1=m32[:], op=AL.subtract)
                    return h, m, l

                mh, ml = split2(flatX, -2.0, "mx")   # -2x
                nh, nl = split2(flatY, -2.0, "my")   # -2y
                xh, xl = split2(flatX, 1.0, "px")    # x
                yh, yl = split2(flatY, 1.0, "py")    # y
                sh, sm, sl = split3sq(flatX, "x")  # x^2
                th, tm, tl = split3sq(flatY, "y")  # y^2
                onesf = ft("onesf", bf16)
                nc.vector.tensor_scalar(
                    out=onesf[:], in0=flatX[:], scalar1=0.0, scalar2=1.0,
                    op0=AL.mult, op1=AL.add,
                )
                o = onesf
                for dst, srcs in (
                    (lhsT1, (mh, mh, ml, ml, sh, sm, sl, o, o, o)),
                    (rhs1, (yh, yl, yh, yl, o, o, o, th, tm, tl)),
                    (lhsT2, (nh, nh, nl, nl, th, tm, tl, o, o, o)),
                    (rhs2, (xh, xl, xh, xl, o, o, o, sh, sm, sl)),
                ):
                    for g, src in enumerate(srcs):
                        rows(dst, g, src)

            # ---- main: two passes over the distance matrix ----
            if scan in ("ttr2", "gps", "none", "quarter"):
                nc.vector.memset(mins1[:], BIG)
                nc.vector.memset(mins2[:], BIG)
            if scan.startswith("tree16") or scan == "evac16":
                assert ntile == 2

            def do_pass(lhsT, rhs, mins):
                if scan == "tree16w":
                    # wide tail tile: per-block halved rows land here, then the
                    # tail tree runs once per pass over all 32 blocks at once
                    HW_ = tree.tile(
                        [P, nblk * (W // 2)], f16, tag="H", bufs=1, name="Hw"
                    )
                    B1 = tree.tile(
                        [P, nblk * (W // 4)], f16, tag="B1", bufs=1, name="B1"
                    )
                    B2 = tree.tile(
                        [P, nblk * (W // 8)], f16, tag="B2", bufs=1, name="B2"
                    )
                elif scan == "tree16x":
                    HW_ = tree.tile(
                        [P, nblk * (W // 4)], f16, tag="H", bufs=1, name="Hw"
                    )
                    B1 = tree.tile(
                        [P, nblk * (W // 8)], f16, tag="B1", bufs=1, name="B1"
                    )
                    B2 = tree.tile(
                        [P, nblk * (W // 16)], f16, tag="B2", bufs=1, name="B2"
                    )
                for i in range(nblk):
                    lw = lhsT[:, i * P : (i + 1) * P]
                    pts = []
                    for t in range(ntile):
                        pt = psum.tile([P, W], f32, tag="pt", name=f"pt_{i}_{t}")
                        for c0 in range(0, W, 512):
                            cw = min(512, W - c0)
                            mm_rhs = rhs[:, t * W + c0 : t * W + c0 + cw]
                            nc.tensor.matmul(
                                pt[:, c0 : c0 + cw], lw, mm_rhs, start=True, stop=True
                            )
                        pts.append(pt)
                    if scan.startswith("tree16") or scan == "evac16":
                        # ACT negate-evacuates both tiles as fp16 (2 elem/cyc);
                        # DVE max-tree on SBUF fp16: TT max (2x) x2, then a
                        # 1x reduce of the quarter-width tail.
                        u = []
                        for t in range(ntile):
                            ut = evac.tile([P, W], f16, tag="ev", name=f"u_{i}_{t}")
                            nc.scalar.mul(ut[:], pts[t][:], -1.0)
                            u.append(ut)
                        if scan == "evac16":
                            continue
                        g = tree.tile([P, W], f16, tag="g", name=f"g_{i}")
                        nc.vector.tensor_tensor(
                            out=g[:], in0=u[0][:], in1=u[1][:], op=AL.max
                        )
                        if scan == "tree16w":
                            # halve g into this block's slot of the wide tile
                            nc.vector.tensor_tensor(
                                out=HW_[:, i * (W // 2) : (i + 1) * (W // 2)],
                                in0=g[:, : W // 2],
                                in1=g[:, W // 2 :],
                                op=AL.max,
                            )
                            continue
                        if scan == "tree16x":
                            # per-block: halve twice more (2D contiguous, 2x),
                            # landing a 256-wide strip in the wide tail tile
                            h = tree.tile([P, W // 2], f16, tag="h", name=f"h_{i}")
                            nc.vector.tensor_tensor(
                                out=h[:], in0=g[:, : W // 2], in1=g[:, W // 2 :],
                                op=AL.max,
                            )
                            nc.vector.tensor_tensor(
                                out=HW_[:, i * (W // 4) : (i + 1) * (W // 4)],
                                in0=h[:, : W // 4],
                                in1=h[:, W // 4 :],
                                op=AL.max,
                            )
                            continue
                        if scan == "tree16nr":
                            continue
                        if scan == "tree16nh":
                            nc.vector.tensor_reduce(
                                out=mins[:, i : i + 1], in_=g[:], axis=AX.X, op=AL.max
                            )
                            continue
                        h = tree.tile([P, W // 2], f16, tag="h", name=f"h_{i}")
                        nc.vector.tensor_tensor(
                            out=h[:], in0=g[:, : W // 2], in1=g[:, W // 2 :], op=AL.max
                        )
                        if scan == "tree16d":
                            # deeper: one more TT level, reduce 512
                            q = tree.tile([P, W // 4], f16, tag="q", name=f"q_{i}")
                            nc.vector.tensor_tensor(
                                out=q[:], in0=h[:, : W // 4], in1=h[:, W // 4 :],
                                op=AL.max,
                            )
                            nc.vector.tensor_reduce(
                                out=mins[:, i : i + 1], in_=q[:], axis=AX.X, op=AL.max
                            )
                            continue
                        if scan == "tree16e":
                            # deepest: TT down to 256, then reduce
                            q = tree.tile([P, W // 4], f16, tag="q", name=f"q_{i}")
                            nc.vector.tensor_tensor(
                                out=q[:], in0=h[:, : W // 4], in1=h[:, W // 4 :],
                                op=AL.max,
                            )
                            q2 = tree.tile([P, W // 8], f16, tag="q2", name=f"q2_{i}")
                            nc.vector.tensor_tensor(
                                out=q2[:], in0=q[:, : W // 8], in1=q[:, W // 8 :],
                                op=AL.max,
                            )
                            nc.vector.tensor_reduce(
                                out=mins[:, i : i + 1], in_=q2[:], axis=AX.X, op=AL.max
                            )
                            continue
                        nc.vector.tensor_reduce(
                            out=mins[:, i : i + 1], in_=h[:], axis=AX.X, op=AL.max
                        )
                    elif scan == "ttr":
                        for k in range(2):
                            s = evac.tile([P, W], f32, tag="ev", name=f"ev_{i}_{k}")
                            nc.scalar.copy(s[:], pts[2 * k + 1][:])
                            scr = evac.tile([P, W], f32, tag="scr", name=f"scr_{i}_{k}")
                            nc.vector.tensor_tensor_reduce(
                                out=scr[:],
                                in0=pts[2 * k][:],
                                in1=s[:],
                                scale=1.0,
                                scalar=BIG,
                                op0=AL.min,
                                op1=AL.min,
                                accum_out=mins[:, 2 * i + k : 2 * i + k + 1],
                            )
                    elif scan == "ttr2":
                        if i % 4 == 0:
                            # direct: DVE min-reduces each PSUM tile
                            for t in range(ntile):
                                nc.vector.tensor_reduce(
                                    out=mins[:, ntile * i + t : ntile * i + t + 1],
                                    in_=pts[t][:],
                                    axis=AX.X,
                                    op=AL.min,
                                )
                        else:
                            # fed: ACT evacuates all four tiles to SBUF, DVE
                            # runs two fused all-SBUF TTR min-scans
                            ss = []
                            for t in range(ntile):
                                s = evac.tile([P, W], f32, tag="ev", name=f"ev_{i}_{t}")
                                nc.scalar.copy(s[:], pts[t][:])
                                ss.append(s)
                            for k in range(ntile // 2):
                                scr = evac.tile(
                                    [P, W], f32, tag="scr", name=f"scr_{i}_{k}"
                                )
                                nc.vector.tensor_tensor_reduce(
                                    out=scr[:],
                                    in0=ss[2 * k][:],
                                    in1=ss[2 * k + 1][:],
                                    scale=1.0,
                                    scalar=BIG,
                                    op0=AL.min,
                                    op1=AL.min,
                                    accum_out=mins[:, ntile * i + k : ntile * i + k + 1],
                                )
                    elif scan == "gps":
                        if i % 4 == 0:
                            for t in range(ntile):
                                nc.vector.tensor_reduce(
                                    out=mins[:, ntile * i + t : ntile * i + t + 1],
                                    in_=pts[t][:],
                                    axis=AX.X,
                                    op=AL.min,
                                )
                        else:
                            # fed: ACT evacuates both tiles, GpSimd halves via
                            # elementwise min, DVE reduces the halved tile
                            ss = []
                            for t in range(2):
                                s = evac.tile([P, W], f32, tag="ev", name=f"ev_{i}_{t}")
                                nc.scalar.copy(s[:], pts[t][:])
                                ss.append(s)
                            g = evac.tile([P, W], f32, tag="gmin", name=f"g_{i}")
                            nc.gpsimd.tensor_tensor(
                                out=g[:], in0=ss[0][:], in1=ss[1][:], op=AL.min
                            )
                            nc.vector.tensor_reduce(
                                out=mins[:, ntile * i : ntile * i + 1],
                                in_=g[:],
                                axis=AX.X,
                                op=AL.min,
                            )
                    elif scan == "none":
                        pass
                    elif scan == "quarter":
                        nc.vector.tensor_reduce(
                            out=mins[:, ntile * i : ntile * i + 1],
                            in_=pts[0][:],
                            axis=AX.X,
                            op=AL.min,
                        )
                    else:
                        for t in range(ntile):
                            nc.vector.tensor_reduce(
                                out=mins[:, ntile * i + t : ntile * i + t + 1],
                                in_=pts[t][:],
                                axis=AX.X,
                                op=AL.min,
                            )

                if scan in ("tree16w", "tree16x"):
                    # batched tail: halve all 32 block strips together via 3D
                    # views [P, nblk, w], down to w=8, then one 1x reduce
                    src, w = HW_, (W // 2 if scan == "tree16w" else W // 4)
                    pp = [B1, B2]
                    k = 0
                    while w > 8:
                        dst = pp[k % 2]
                        sv = src[:, : nblk * w].rearrange(
                            "p (b w) -> p b w", w=w
                        )
                        dv = dst[:, : nblk * (w // 2)].rearrange(
                            "p (b w) -> p b w", w=w // 2
                        )
                        nc.vector.tensor_tensor(
                            out=dv,
                            in0=sv[:, :, : w // 2],
                            in1=sv[:, :, w // 2 :],
                            op=AL.max,
                        )
                        src, w = dst, w // 2
                        k += 1
                    fv = src[:, : nblk * 8].rearrange("p (b w) -> p b w", w=8)
                    nc.vector.tensor_reduce(
                        out=mins[:], in_=fv, axis=AX.X, op=AL.max
                    )

            def body():
                do_pass(lhsT1, rhs1, mins1)
                do_pass(lhsT2, rhs2, mins2)
                # per-row-block min combine, then sum over blocks
                for pi, (mins, minb) in enumerate(((mins1, minb1), (mins2, minb2))):
                    if scan.startswith("tree16") or scan == "evac16":
                        # mins holds max(-t) = -blockmin; sum then negate
                        nc.vector.reduce_sum(
                            out=outt[:, pi : pi + 1], in_=mins[:], axis=AX.X
                        )
                        nc.vector.tensor_scalar_mul(
                            out=outt[:, pi : pi + 1],
                            in0=outt[:, pi : pi + 1],
                            scalar1=-1.0,
                        )
                        continue
                    mv = mins[:].rearrange("p (i k) -> p i k", k=nacc)
                    nc.vector.tensor_reduce(out=minb[:], in_=mv, axis=AX.X, op=AL.min)
                    nc.vector.reduce_sum(
                        out=outt[:, pi : pi + 1], in_=minb[:], axis=AX.X
                    )

            if reps == 1:
                body()
            else:
                # benchmark mode: repeat the whole compute to make the kernel
                # long enough for wall-clock timing
                with tc.For_i(0, reps, 1):
                    body()

            nc.sync.dma_start(out=out_d[:], in_=outt[:])

    nc.compile()
    return nc


# Best hardware-validated configuration: bf16 hi/lo decomposition matmuls
# (fp32-accurate, loss rel err ~1e-5) with the fp16 max-tree scan: ACT
# negate-evacuates each PSUM tile to fp16 SBUF at 2 elem/cycle (fp16-out
# Accel=2), then DVE runs a 4-level TT-max tree (2x_1P mode, 2 results/cycle)
# plus a short 1x reduce tail. ~1.26x over the direct DVE min-reduce scan,
# which is pinned at 1 elem/cycle (tensor_reduce has no fast perf modes; the
# fused TTR min- AND max-reduce ucode paths both fault on TRN2 hardware, and
# tensor_tensor_scan's recurrence runs at 2 cycles/step = no gain).
# fp16 rounding of the (negated) distances keeps relative error ~5e-4 per
# element; measured loss rel err ~1e-5.
BEST = dict(mm_dtype="bf16x", scan="tree16e", ntile=2, evac_bufs=8, tree_bufs=6)


def _program(**kw):
    cfg = dict(BEST)
    cfg.update(kw)
    key = tuple(sorted(cfg.items()))
    if key not in _cache:
        _cache[key] = _build(**cfg)
    return _cache[key]


def kernel(X, Y, ps=None, **kw):
    from concourse.bass_utils import run_bass_kernel_spmd

    X = np.asarray(X, dtype=np.float32)
    Y = np.asarray(Y, dtype=np.float32)
    assert X.shape == (B, C, N) and Y.shape == (B, C, N)

    nc = _program()
    in_maps = [
        {"X": np.ascontiguousarray(X[b]), "Y": np.ascontiguousarray(Y[b])}
        for b in range(B)
    ]
    res = run_bass_kernel_spmd(nc, in_maps, list(range(B)))
    total = 0.0
    for r in res.results:
        total += r["out"].astype(np.float64).sum()
    return np.float32(total / (2.0 * B * N))

